# revision 9
# baseline (speedup 1.0000x reference)
"""Trainium2 Bass kernel for windowed multi-head attention with conv QKV.

Shapes (hardcoded): x (2,64,32,192), D_MODEL=32, N_HEADS=8, c=4, QS=24,
FLANGE=8, F=40, T=192, M=8 blocks. 8 NeuronCores.

Sharding: core k owns batch b=k//4 and block pair m0=2*(k%4); it computes
all 8 heads for its two blocks (16 attention groups) plus its slice of the
final conv (second launch).
"""

import numpy as np
import concourse.bass as bass
import concourse.bacc as bacc
import concourse.mybir as mybir
from concourse.tile import TileContext
from concourse.bass_utils import run_bass_kernel_spmd

F32 = mybir.dt.float32
F32R = mybir.dt.float32r
U32 = mybir.dt.uint32
AF = mybir.ActivationFunctionType

NCORES = 8
B, CIN, H, W = 2, 64, 32, 192
DM, NH, CH = 32, 8, 4          # d_model, heads, depth/head
QS, FL, F = 24, 8, 40          # query block, flange, window
M = W // QS                    # 8 blocks
POS = B * H * W                # 12288
PB = H * W                     # 6144 positions per batch
NPAD = B * (H + 2) * W         # 13056 (h-padded raster)
ROWP = (H + 2) * W             # 6528
KFSZ = B * NH * CH * H * (W + 2 * FL)   # 425984 padded k storage
# buggy as_strided strides (elements) over the padded storage
S_B, S_N, S_C, S_H = NH * CH * H * W, CH * H * W, H * W, W  # 196608,24576,6144,192
HF = H * F                     # 1280 keys per group
HQ = H * QS                    # 768 queries per group

_CACHE = {}


def _sap(tile, p0, npart, off, dims):
    """Custom strided view of an SBUF/PSUM pool tile.

    Partition step comes from the tile's own AP (accounts for allocation
    padding); `off` is a free-dim element offset, `dims` the free dims.
    """
    b0 = tile[:]
    ps = int(b0.ap[0][0])
    return bass.AP(b0.tensor, b0.offset + p0 * ps + off, [[ps, npart]] + dims)



def _build_launch1():
    nc = bacc.Bacc(None, target_bir_lowering=False, debug=False,
                   num_devices=NCORES)
    x = nc.dram_tensor("x", [B, CIN, H, W], F32R, kind="ExternalInput").ap()
    wA = [nc.dram_tensor(f"wA{dy}", [128, 96], F32R, kind="ExternalInput").ap()
          for dy in range(3)]
    wB = [nc.dram_tensor(f"wB{dy}", [64, 96], F32R, kind="ExternalInput").ap()
          for dy in range(3)]
    cbias = nc.dram_tensor("cbias", [96, 1], F32, kind="ExternalInput").ap()
    zpad = nc.dram_tensor("zpad", [B, 2, 32, 32, 8], F32R,
                          kind="ExternalInput").ap()
    id4 = nc.dram_tensor("id4", [4, 4], F32R, kind="ExternalInput").ap()
    zf = nc.dram_tensor("zf", [128, 384], F32R, kind="ExternalInput").ap()
    v2init = nc.dram_tensor("v2init", [128, 330], F32R,
                            kind="ExternalInput").ap()
    par = nc.dram_tensor("par", [1, 4], U32, kind="ExternalInput").ap()
    o_out = nc.dram_tensor("o_out", [DM, H, 2 * QS], F32,
                           kind="ExternalOutput").ap()

    kf = nc.dram_tensor("kf", [KFSZ], F32R).ap()
    vf = nc.dram_tensor("vf", [KFSZ], F32R).ap()
    qfull = nc.dram_tensor("qfull", [DM, POS], F32R).ap()

    with TileContext(nc) as tc:
        # ---------------- conv phase ----------------
        with (
            tc.tile_pool(name="xw", bufs=1) as xw,
            tc.tile_pool(name="ystage", bufs=1) as ystage,
            tc.tile_pool(name="cps", bufs=2, space="PSUM") as cps,
        ):
            wA_sb = [xw.tile([128, 96], F32R, tag=f"wA{dy}", name=f"wA{dy}sb") for dy in range(3)]
            wB_sb = [xw.tile([64, 96], F32R, tag=f"wB{dy}", name=f"wB{dy}sb") for dy in range(3)]
            bias_sb = xw.tile([96, 1], F32, tag="cbias", name="cbias")
            nc.sync.dma_start(out=bias_sb[:], in_=cbias[:])
            for dy in range(3):
                nc.sync.dma_start(out=wA_sb[dy][:], in_=wA[dy][:])
                nc.sync.dma_start(out=wB_sb[dy][:], in_=wB[dy][:])
            id_sb = xw.tile([4, 4], F32R, tag="id4", name="id4")
            nc.sync.dma_start(out=id_sb[:], in_=id4[:])
            par_sb = xw.tile([1, 4], U32, tag="par", name="par")
            nc.sync.dma_start(out=par_sb[:], in_=par[:])

            # X2: rows 0-63 dx=-1, rows 64-127 dx=0 ; X1: rows 0-63 dx=+1,
            # row 64 ones (bias). free = (b, hh in [0,34), w)
            X2 = xw.tile([128, NPAD], F32R, tag="X2", name="X2")
            X1 = xw.tile([64, NPAD], F32R, tag="X1", name="X1")
            # zero pad rows (hh=0, hh=33 per b) and shifted edge cols
            for hh_off in (0, (H + 1) * W):
                nc.sync.dma_start(
                    out=_sap(X2, 0, 128, hh_off, [[ROWP, 2], [1, W]]),
                    in_=bass.AP(zf.tensor, 0, [[384, 128], [W, 2], [1, W]]))
                nc.sync.dma_start(
                    out=_sap(X1, 0, 64, hh_off, [[ROWP, 2], [1, W]]),
                    in_=bass.AP(zf.tensor, 0, [[384, 64], [W, 2], [1, W]]))
            nc.sync.dma_start(
                out=_sap(X2, 0, 64, 0, [[W, 2 * (H + 2)], [1, 1]]),
                in_=bass.AP(zf.tensor, 0, [[384, 64], [1, 2 * (H + 2)], [1, 1]]))
            nc.sync.dma_start(
                out=_sap(X1, 0, 64, W - 1, [[W, 2 * (H + 2)], [1, 1]]),
                in_=bass.AP(zf.tensor, 0, [[384, 64], [1, 2 * (H + 2)], [1, 1]]))
            for b in range(B):
                xb = x[b]  # (64, 32, 192)
                # dx=-1: dest w 1..191 <- src w 0..190
                nc.sync.dma_start(
                    out=_sap(X2, 0, 64, b * ROWP + W + 1,
                             [[W, H], [1, W - 1]]),
                    in_=bass.AP(xb.tensor, xb.offset,
                                [[PB, 64], [W, H], [1, W - 1]]))
                # dx=0
                nc.sync.dma_start(
                    out=_sap(X2, 64, 64, b * ROWP + W, [[W, H], [1, W]]),
                    in_=bass.AP(xb.tensor, xb.offset,
                                [[PB, 64], [W, H], [1, W]]))
                # dx=+1: dest w 0..190 <- src w 1..191
                nc.sync.dma_start(
                    out=_sap(X1, 0, 64, b * ROWP + W, [[W, H], [1, W - 1]]),
                    in_=bass.AP(xb.tensor, xb.offset + 1,
                                [[PB, 64], [W, H], [1, W - 1]]))

            Y = ystage.tile([96, POS], F32R, tag="Y", name="Y")
            NT = 512
            for b in range(B):
                for ct in range(PB // NT):
                    yp = cps.tile([96, NT], F32, tag="yp", name="yp")
                    p0 = ct * NT
                    for dy in range(3):
                        off = b * ROWP + dy * W + p0
                        nc.tensor.matmul(
                            yp[:], wA_sb[dy][:],
                            _sap(X2, 0, 128, off, [[1, NT]]),
                            start=(dy == 0), stop=False)
                    for dy in range(3):
                        off = b * ROWP + dy * W + p0
                        nc.tensor.matmul(
                            yp[:], wB_sb[dy][:],
                            _sap(X1, 0, 64, off, [[1, NT]]),
                            start=False, stop=(dy == 2))
                    dst = b * PB + p0
                    nc.vector.tensor_scalar_add(Y[:, dst:dst + NT], yp[:],
                                                bias_sb[:])

            # write q_full / padded k_flat,v_flat to DRAM
            nc.sync.dma_start(out=qfull[:], in_=Y[0:32, :])
            W2 = W + 2 * FL   # 208 storage row
            for b in range(B):
                for (prow, dst) in ((32, kf), (64, vf)):
                    nc.sync.dma_start(
                        out=bass.AP(dst.tensor, b * 32 * H * W2 + FL,
                                    [[H * W2, 32], [W2, H], [1, W]]),
                        in_=Y[prow:prow + 32, b * PB:(b + 1) * PB]
                        .rearrange("p (h w) -> p h w", h=H))
                    for fi, fo in ((0, 0), (1, W2 - FL)):
                        zp = zpad[b, fi]  # (32, 32, 8)
                        nc.sync.dma_start(
                            out=bass.AP(dst.tensor, b * 32 * H * W2 + fo,
                                        [[H * W2, 32], [W2, H], [1, FL]]),
                            in_=bass.AP(zp.tensor, zp.offset,
                                        [[H * FL, 32], [FL, H], [1, FL]]))

            # base registers for dynamic gathers (SP engine)
            r0 = nc.sync.alloc_register("cb")
            nc.sync.reg_load(r0, par_sb[0:1, 0:1])
            cbase = nc.snap(r0, min_val=0, max_val=300000)
            r1 = nc.sync.alloc_register("qb")
            nc.sync.reg_load(r1, par_sb[0:1, 1:2])
            qbase = nc.snap(r1, min_val=0, max_val=300000)

        tc.strict_bb_all_engine_barrier()

        # ---------------- attention phase ----------------
        with (
            tc.tile_pool(name="ga", bufs=2) as ga,
            tc.tile_pool(name="ste", bufs=3) as stp,
            tc.tile_pool(name="ops", bufs=1, space="PSUM") as ops,
            tc.tile_pool(name="pst", bufs=2, space="PSUM") as pst,
            tc.tile_pool(name="on", bufs=2) as on,
        ):
            # persistent v2e staging (ones at col t*33+32, zeros elsewhere)
            v2e = [ga.tile([128, 330], F32R, tag=f"v2e{i}", name=f"v2e{i}") for i in range(2)]
            for t in v2e:
                nc.sync.dma_start(out=t[:], in_=v2init[:])

            for n in range(NH):
                for mm in range(2):
                    g = n * 2 + mm
                    gb = cbase + (n * S_N + mm * QS)
                    qb = qbase + (n * CH * POS + mm * QS)

                    k2 = ga.tile([4, HF], F32R, tag="k2", name="k2")
                    nc.sync.dma_start(
                        out=k2[:],
                        in_=bass.AP(kf.tensor, gb,
                                    [[S_C, 4], [S_H, H], [1, F]]))
                    v2k = ga.tile([4, HF], F32R, tag="v2k", name="v2k")
                    nc.sync.dma_start(
                        out=v2k[:],
                        in_=bass.AP(vf.tensor, gb,
                                    [[S_C, 4], [S_H, H], [1, F]]))
                    qg = ga.tile([4, HQ], F32R, tag="qg", name="qg")
                    nc.sync.dma_start(
                        out=qg[:],
                        in_=bass.AP(qfull.tensor, qb,
                                    [[POS, 4], [W, H], [1, QS]]))

                    # transpose v2k -> (128,40) per 128-key tile, pack v2e
                    ve = v2e[g % 2]
                    vt = pst.tile([128, 40], F32R, tag="stq", name="stq")
                    for kt in range(10):
                        nc.tensor.transpose(
                            vt[:, kt * 4:(kt + 1) * 4],
                            v2k[:, kt * 128:(kt + 1) * 128], id_sb[:])
                    nc.vector.tensor_copy(
                        _sap(ve, 0, 128, 0, [[33, 10], [1, 4]]),
                        _sap(vt, 0, 128, 0, [[4, 10], [1, 4]]))

                    oA = ops.tile([33, 512], F32, tag="oA", name="oA")
                    oB = ops.tile([33, 256], F32, tag="oB", name="oB")
                    for st_i in range(5):
                        st = pst.tile([128, 1536], F32, tag="stq", name="stq")
                        # bank-aligned writes: [512@0, 256@512, 256@768, 512@1024]
                        kt0 = st_i * 2
                        kt1 = kt0 + 1
                        nc.tensor.matmul(
                            st[:, 0:512], k2[:, kt0 * 128:(kt0 + 1) * 128],
                            qg[:, 0:512],
                            start=True, stop=True, skip_group_check=True)
                        nc.tensor.matmul(
                            st[:, 512:768], k2[:, kt0 * 128:(kt0 + 1) * 128],
                            qg[:, 512:768],
                            start=True, stop=True, skip_group_check=True)
                        nc.tensor.matmul(
                            st[:, 768:1024], k2[:, kt1 * 128:(kt1 + 1) * 128],
                            qg[:, 0:256],
                            start=True, stop=True, skip_group_check=True)
                        nc.tensor.matmul(
                            st[:, 1024:1536], k2[:, kt1 * 128:(kt1 + 1) * 128],
                            qg[:, 256:768],
                            start=True, stop=True, skip_group_check=True)
                        ste = stp.tile([128, 1536], F32R, tag="ste", name="ste")
                        nc.scalar.activation(ste[:], st[:], AF.Exp)
                        for half in range(2):
                            kt = st_i * 2 + half
                            c0 = half * 768
                            lhs = _sap(ve, 0, 128, kt * 33, [[1, 33]])
                            nc.tensor.matmul(
                                oA[:], lhs, ste[:, c0:c0 + 512],
                                start=(kt == 0), stop=(kt == 9),
                                skip_group_check=True)
                            nc.tensor.matmul(
                                oB[:], lhs, ste[:, c0 + 512:c0 + 768],
                                start=(kt == 0), stop=(kt == 9),
                                skip_group_check=True)

                    rec = on.tile([1, HQ], F32, tag="rec", name="rec")
                    nc.vector.reciprocal(rec[:, 0:512], oA[32:33, :])
                    nc.vector.reciprocal(rec[:, 512:768], oB[32:33, :])
                    rec4 = on.tile([4, HQ], F32, tag="rec4", name="rec4")
                    nc.gpsimd.partition_broadcast(rec4[:], rec[:])
                    o4 = on.tile([4, HQ], F32, tag="o4", name="o4")
                    nc.vector.tensor_mul(o4[:, 0:512], oA[0:4, :],
                                         rec4[:, 0:512])
                    nc.vector.tensor_mul(o4[:, 512:768], oB[0:4, :],
                                         rec4[:, 512:768])
                    nc.sync.dma_start(
                        out=bass.AP(o_out.tensor,
                                    n * 4 * H * 2 * QS + mm * QS,
                                    [[H * 2 * QS, 4], [2 * QS, H], [1, QS]]),
                        in_=_sap(o4, 0, 4, 0, [[QS, H], [1, QS]]))

    nc.finalize()
    return nc


def _build_launch2():
    nc = bacc.Bacc(None, target_bir_lowering=False, debug=False,
                   num_devices=NCORES)
    WH = 2 * QS + 2  # 50 cols with halo
    oh = nc.dram_tensor("oh", [DM, H + 2, WH], F32R, kind="ExternalInput").ap()
    w2 = [nc.dram_tensor(f"w2{dy}", [96, 64], F32R, kind="ExternalInput").ap()
          for dy in range(3)]
    z32 = nc.dram_tensor("z32", [32, 1], F32R, kind="ExternalInput").ap()
    out = nc.dram_tensor("out", [64, H * 2 * QS], F32,
                         kind="ExternalOutput").ap()
    NPAD2 = (H + 2) * WH  # 1700

    with TileContext(nc) as tc:
        with (
            tc.tile_pool(name="sb", bufs=1) as sb,
            tc.tile_pool(name="ps", bufs=2, space="PSUM") as ps,
        ):
            w2_sb = [sb.tile([96, 64], F32R, tag=f"w2{dy}", name=f"w2{dy}sb") for dy in range(3)]
            for dy in range(3):
                nc.sync.dma_start(out=w2_sb[dy][:], in_=w2[dy][:])
            osb = sb.tile([32, NPAD2], F32R, tag="osb", name="osb")
            nc.sync.dma_start(out=osb[:], in_=oh[:].rearrange("c h w -> c (h w)"))
            osh = sb.tile([96, NPAD2], F32R, tag="osh", name="osh")
            nc.sync.dma_start(out=osh[0:32, 0:1], in_=z32[:])
            nc.sync.dma_start(out=osh[64:96, NPAD2 - 1:NPAD2], in_=z32[:])
            nc.vector.tensor_copy(osh[0:32, 1:NPAD2], osb[:, 0:NPAD2 - 1])
            nc.vector.tensor_copy(osh[32:64, :], osb[:])
            nc.vector.tensor_copy(osh[64:96, 0:NPAD2 - 1], osb[:, 1:NPAD2])

            ot = sb.tile([64, H * 2 * QS], F32, tag="ot", name="ot")
            hsz = [10, 10, 10, 2]
            h0 = 0
            for hi, hn in enumerate(hsz):
                nt = hn * WH
                yp = ps.tile([64, 500], F32, tag="yp", name="yp")
                for dy in range(3):
                    off = (h0 + dy) * WH
                    nc.tensor.matmul(
                        yp[:, 0:nt], w2_sb[dy][:],
                        _sap(osh, 0, 96, off, [[1, nt]]),
                        start=(dy == 0), stop=(dy == 2))
                nc.vector.tensor_copy(
                    _sap(ot, 0, 64, h0 * 2 * QS, [[2 * QS, hn], [1, 2 * QS]]),
                    _sap(yp, 0, 64, 1, [[WH, hn], [1, 2 * QS]]))
                h0 += hn
            nc.sync.dma_start(out=out[:], in_=ot[:])
    nc.finalize()
    return nc


def _round_f32r(a):
    return a.astype(np.float32)


def _prep_qkv_weights(q_w, q_b, k_w, k_b, v_w, v_b):
    # fold attention scale into q
    sc = CH ** -0.5
    q_w = q_w * sc
    q_b = q_b * sc
    Wc = np.concatenate([q_w, k_w, v_w], axis=0)   # (96, 64, 3, 3)
    bc = np.concatenate([q_b, k_b, v_b], axis=0)   # (96,)
    wA, wB = [], []
    for dy in range(3):
        a = np.zeros((128, 96), np.float32)
        a[0:64, :] = Wc[:, :, dy, 0].T    # dx=-1
        a[64:128, :] = Wc[:, :, dy, 1].T  # dx=0
        wA.append(a)
        b = Wc[:, :, dy, 2].T.copy()    # dx=+1
        wB.append(b)
    return wA, wB, bc.reshape(96, 1)


def kernel(x, q_w, q_b, k_w, k_b, v_w, v_b, out_w):
    x = np.asarray(x, np.float32)
    if "l1" not in _CACHE:
        _CACHE["l1"] = _build_launch1()
        _CACHE["l2"] = _build_launch2()
    nc1, nc2 = _CACHE["l1"], _CACHE["l2"]

    wA, wB, cbias = _prep_qkv_weights(np.asarray(q_w, np.float32), np.asarray(q_b, np.float32),
                               np.asarray(k_w, np.float32), np.asarray(k_b, np.float32),
                               np.asarray(v_w, np.float32), np.asarray(v_b, np.float32))
    zpad = np.zeros((B, 2, 32, 32, 8), np.float32)
    id4 = np.eye(4, dtype=np.float32)
    zf = np.zeros((128, 384), np.float32)
    v2init = np.zeros((128, 330), np.float32)
    v2init[:, 32::33] = 1.0
    in_maps = []
    for k in range(NCORES):
        b, m0 = k // 4, 2 * (k % 4)
        par = np.array([[b * S_B + m0 * QS, b * PB + m0 * QS, 0, 0]], np.uint32)
        m = {"x": x, "zpad": zpad, "id4": id4, "par": par, "cbias": cbias, "zf": zf, "v2init": v2init}
        for dy in range(3):
            m[f"wA{dy}"] = wA[dy]
            m[f"wB{dy}"] = wB[dy]
        in_maps.append(m)
    res1 = run_bass_kernel_spmd(nc1, in_maps, list(range(NCORES)))

    # assemble o (B, 32, H, W)
    o = np.zeros((B, DM, H, W), np.float32)
    for k in range(NCORES):
        b, m0 = k // 4, 2 * (k % 4)
        o[b, :, :, m0 * QS:(m0 + 2) * QS] = res1.results[k]["o_out"]

    # launch 2: output conv, sharded by (b, column pair)
    w2 = []
    ow = np.asarray(out_w, np.float32)
    for dy in range(3):
        a = np.zeros((96, 64), np.float32)
        for dx in range(3):
            a[dx * 32:(dx + 1) * 32, :] = ow[:, :, dy, dx].T
        w2.append(a)
    in_maps2 = []
    WH = 2 * QS + 2
    for k in range(NCORES):
        b, m0 = k // 4, 2 * (k % 4)
        ohal = np.zeros((DM, H + 2, WH), np.float32)
        c0 = m0 * QS
        lo, hi = max(0, c0 - 1), min(W, c0 + 2 * QS + 1)
        ohal[:, 1:H + 1, (lo - (c0 - 1)):(hi - (c0 - 1))] = o[b, :, :, lo:hi]
        mm = {"oh": ohal, "z32": np.zeros((32, 1), np.float32)}
        for dy in range(3):
            mm[f"w2{dy}"] = w2[dy]
        in_maps2.append(mm)
    res2 = run_bass_kernel_spmd(nc2, in_maps2, list(range(NCORES)))

    out = np.zeros((B, 64, H, W), np.float32)
    for k in range(NCORES):
        b, m0 = k // 4, 2 * (k % 4)
        out[b, :, :, m0 * QS:(m0 + 2) * QS] = \
            res2.results[k]["out"].reshape(64, H, 2 * QS)
    return out
